# revision 41
# baseline (speedup 1.0000x reference)
"""Banded (sliding-window) GQA attention block on 8 trn2 cores.

Sharding: 8 cores = batch(4) x seq-halves(2). Each core computes 1024
queries for one batch element with a 128-position K/V halo on each side
(window half = 127, padded to 128 so everything is 128-aligned).

Layouts are transposed ([feature, seq]) so the tensor engine contracts
naturally. RoPE even/odd lanes are packed per head into 64 contiguous
partitions ([e0..e31, o0..o31]) so each score block is a single K=64
matmul; the rotation is computed as P1*C + P2*S' where P2 is a 32-row
pair-swap of the projection PSUM obtained with one permutation matmul.

Band masking multiplies the bf16 probabilities with a 0/1 band tile on
the vector engine (2x 16-bit mode) instead of a -inf matmul. Softmax
denominators come from an appended ones-column in V; no max-subtraction
(scores are small enough that raw exp fits in f32).
"""

import sys

sys.path.insert(0, "/opt/trn_rl_repo")

import numpy as np

import concourse.bass as bass
from concourse import bacc
import concourse.mybir as mybir
import concourse.tile as tile
from concourse.bass_utils import run_bass_kernel_spmd
from concourse.masks import make_identity

B, S, D = 4, 2048, 1024
H, KVH, HD = 16, 2, 64
W, HWD = 255, 127
SL = S // 2              # local queries per core
PAD = 128                # left/right key padding (>= half window, 128-aligned)
U = SL + 2 * PAD         # 1280 padded key columns
UQ = U + 256             # 1536: query tensors padded 128 each side
NCH = U // 128           # 10 key chunks

f32 = mybir.dt.float32
f32r = mybir.dt.float32r
bf16 = mybir.dt.bfloat16

Exp = mybir.ActivationFunctionType.Exp
Ident = mybir.ActivationFunctionType.Identity
MULT = mybir.AluOpType.mult
ADD = mybir.AluOpType.add
DIV = mybir.AluOpType.divide




def tile_ctx(tc, name, bufs):
    return tc.tile_pool(name=name, bufs=bufs)

def build_nc():
    nc = bacc.Bacc("TRN2")
    dp = nc.declare_dram_parameter
    xT = dp("xT", [D, U], bf16, isOutput=False)
    wqm = dp("wqm", [128, 8192], bf16, isOutput=False)
    wkm = dp("wkm", [128, 1024], bf16, isOutput=False)
    wvm = dp("wvm", [128, 1024], bf16, isOutput=False)
    wom = dp("wom", [D, D], bf16, isOutput=False)
    cqt = dp("cqt", [128, SL], f32, isOutput=False)
    sqt = dp("sqt", [128, SL], f32, isOutput=False)
    ckt = dp("ckt", [128, U], f32, isOutput=False)
    skt = dp("skt", [128, U], f32, isOutput=False)
    permm = dp("permm", [128, 128], f32r, isOutput=False)
    bandm = dp("bandm", [128, 768], bf16, isOutput=False)
    bqc = dp("bqc", [128, 8], f32, isOutput=False)
    bkc = dp("bkc", [128, 1], f32, isOutput=False)
    bvc = dp("bvc", [128, 1], f32, isOutput=False)
    boc = dp("boc", [1, D], f32r, isOutput=False)
    out = dp("out", [SL, D], f32, isOutput=True)

    NB = [(0, 512), (512, 512), (1024, 256)]  # N-blocks over U
    NBQ = [(0, 512), (512, 512)]              # N-blocks over SL

    with tile.TileContext(nc) as tc:
        with (
            nc.allow_low_precision(reason="f32r tiles are 4-byte; elementwise ops only"),
            tc.tile_pool(name="persist", bufs=1) as pe,
        ):
            # ---- persistent SBUF ----
            ident_f = pe.tile([128, 128], f32, tag="identf")
            make_identity(nc, ident_f)
            ident = pe.tile([128, 128], f32r, tag="ident")
            nc.vector.tensor_copy(ident[:], ident_f[:])
            ones_f = pe.tile([1, 512], f32, tag="onesf")
            nc.vector.memset(ones_f[:], 1.0)
            ones = pe.tile([1, 512], f32r, tag="ones")
            nc.vector.tensor_copy(ones[:], ones_f[:])

            # small parameter tiles (issue DMAs early; tiny transfers)
            bvc_sb = pe.tile([128, 1], f32, tag="bvc")
            bkc_sb = pe.tile([128, 1], f32, tag="bkc")
            bqc_sb = pe.tile([128, 8], f32, tag="bqc")
            bo_sb = pe.tile([1, D], f32r, tag="bo")
            nc.sync.dma_start(bvc_sb[:], bvc[:])
            nc.sync.dma_start(bkc_sb[:], bkc[:])
            nc.sync.dma_start(bqc_sb[:], bqc[:])
            nc.sync.dma_start(bo_sb[:], boc[:])

            qm = [pe.tile([128, UQ], f32r, tag=f"qm{t}", name=f"qm{t}") for t in range(8)]
            ktr = pe.tile([128, U], f32r, tag="ktr")
            vaug = [pe.tile([128, 65 * NCH], bf16, tag=f"vaug{k}", name=f"vaug{k}") for k in range(2)]
            for k in range(2):
                nc.vector.memset(vaug[k][:], 1.0)
            # zero the query padding wings (only cols 256:1280 get written)
            for t in range(8):
                nc.vector.memset(qm[t][:, 0:256], 0.0)
                nc.vector.memset(qm[t][:, UQ - 256 : UQ], 0.0)

            perm_sb = pe.tile([128, 128], f32r, tag="perm")
            band = pe.tile([128, 768], bf16, tag="band")

            # ================= phase A: projections + rope =================
            with (
                tc.tile_pool(name="proj_in", bufs=1) as pin,
                tc.tile_pool(name="ptmp", bufs=1) as ptmp,
            ):
                wv_sb = pin.tile([128, 1024], bf16, tag="wv")
                xts = [pin.tile([128, U], bf16, tag=f"x{i}", name=f"x{i}") for i in range(8)]
                wkm_sb = pin.tile([128, 1024], bf16, tag="wkm")
                wqm_sb = pin.tile([128, 8192], bf16, tag="wqm")
                cq_sb = pin.tile([128, SL], f32, tag="cq")
                sq_sb = pin.tile([128, SL], f32, tag="sq")
                ck_sb = pin.tile([128, U], f32, tag="ck")
                sk_sb = pin.tile([128, U], f32, tag="sk")

                # DMA issues spread over three queues (SP / Pool / Act), each
                # in just-in-time consumption order: x gates everything, then
                # the V/K weights, then per-tile Q weights, trig mid-stream.
                v_kc = lambda kc: slice(128 * kc, 128 * kc + 128)
                nc.scalar.dma_start(wv_sb[:], wvm[:])
                for i in range(0, 8, 2):
                    nc.sync.dma_start(xts[i][:], xT[v_kc(i), :])
                    nc.gpsimd.dma_start(xts[i + 1][:], xT[v_kc(i + 1), :])
                nc.scalar.dma_start(wkm_sb[:], wkm[:])
                nc.scalar.dma_start(perm_sb[:], permm[:])
                for i in range(8):
                    q = nc.gpsimd if i % 2 else nc.sync
                    q.dma_start(
                        wqm_sb[:, 1024 * i : 1024 * i + 1024], wqm[:, 1024 * i : 1024 * i + 1024]
                    )
                    if i == 2:
                        nc.scalar.dma_start(ck_sb[:], ckt[:])
                        nc.scalar.dma_start(sk_sb[:], skt[:])
                    if i == 5:
                        nc.scalar.dma_start(cq_sb[:], cqt[:])
                        nc.scalar.dma_start(sq_sb[:], sqt[:])
                nc.scalar.dma_start(band[:], bandm[:])

                # transpose psum pool opened FIRST so it gets banks nothing
                # else has touched (avoids false WARs on psum reuse)
                tps_ctx = tc.tile_pool(name="tps", bufs=2, space="PSUM")
                tps = tps_ctx.__enter__()

                # ---- V projection, transposed: Vt[vd, seq] then per-chunk T ----
                vt_ps, free_vt = tc.tile([128, U], f32, space="PSUM", name="vt_ps")
                for n0, nw in NB:
                    for kc in range(8):
                        nc.tensor.matmul(
                            vt_ps[:, n0 : n0 + nw],
                            wv_sb[:, v_kc(kc)],
                            xts[kc][:, n0 : n0 + nw],
                            start=(kc == 0),
                            stop=(kc == 7),
                        )
                vt_sb = pin.tile([128, U], f32r, tag="vts")
                nc.scalar.activation(vt_sb[:], vt_ps[:], Ident, bias=bvc_sb[:, 0:1])
                free_vt()

                # ---- K projection mms (paced by the xts DMAs anyway) ----
                p1k, free_p1k = tc.tile([128, U], f32, space="PSUM", name="p1k")
                for n0, nw in NB:
                    for kc in range(8):
                        nc.tensor.matmul(
                            p1k[:, n0 : n0 + nw],
                            wkm_sb[:, v_kc(kc)],
                            xts[kc][:, n0 : n0 + nw],
                            start=(kc == 0),
                            stop=(kc == 7),
                        )
                p1k_sb = pin.tile([128, U], f32r, tag="p1ks")
                nc.scalar.activation(p1k_sb[:], p1k[:], Ident, bias=bkc_sb[:, 0:1])
                free_p1k()

                # ---- V transposes; per (group, kv) batched vaug copies ----
                for g in range(3):
                    tp = tps.tile([128, 512], f32r, tag="tp")
                    ns_ = 8 if g < 2 else 4
                    for s in range(ns_):
                        st, kvh = (8 * g + s) // 2, (8 * g + s) % 2
                        isl = slice(64 * kvh, 64 * kvh + 64)
                        nc.tensor.matmul(
                            tp[:, 64 * s : 64 * s + 64],
                            vt_sb[isl, 128 * st : 128 * st + 128],
                            ident[isl, isl],
                            is_transpose=True,
                        )
                    for kvh in range(2):
                        nv = ns_ // 2
                        src = tp[:, 64 * kvh : 64 * kvh + 512 - 64].rearrange(
                            "p (s f) -> p s f", f=64
                        )[:, 0 : 2 * nv - 1 : 2, :]
                        dst = vaug[kvh][:, 65 * 4 * g : 65 * 4 * g + 65 * nv].rearrange(
                            "p (s f) -> p s f", f=65
                        )[:, :, 0:64]
                        nc.scalar.copy(dst, src)
                tps_ctx.__exit__(None, None, None)

                # ---- Q0 projection mms ahead of the K tail ----
                def q_proj_mms(t, p1):
                    for n0, nw in NBQ:
                        for kc in range(8):
                            nc.tensor.matmul(
                                p1[:, n0 : n0 + nw],
                                wqm_sb[:, 1024 * t + 128 * kc : 1024 * t + 128 * kc + 128],
                                xts[kc][:, 128 + n0 : 128 + n0 + nw],
                                start=(kc == 0),
                                stop=(kc == 7),
                            )

                def q_copy(t, p1):
                    p1_sb = ptmp.tile([128, SL], f32r, tag="p1s")
                    nc.scalar.activation(p1_sb[:], p1[:], Ident, bias=bqc_sb[:, t : t + 1])
                    return p1_sb

                def q_tail(t, p1_sb, p2_alloc):
                    p2 = p2_alloc()
                    for n0, nw in NBQ:
                        nc.tensor.matmul(
                            p2[:, n0 : n0 + nw], perm_sb[:], p1_sb[:, n0 : n0 + nw],
                            start=True, stop=True,
                        )
                    p2_sb = ptmp.tile([128, SL], f32r, tag="p2s")
                    nc.vector.tensor_copy(p2_sb[:], p2[:])
                    nc.vector.tensor_tensor(p1_sb[:], p1_sb[:], cq_sb[:], MULT)
                    nc.vector.tensor_tensor(p2_sb[:], p2_sb[:], sq_sb[:], MULT)
                    nc.vector.tensor_tensor(qm[t][:, 256 : 256 + SL], p1_sb[:], p2_sb[:], ADD)

                p1_t0, free_q0p1 = tc.tile([128, SL], f32, space="PSUM", name="q0p1")
                q_proj_mms(0, p1_t0)

                # ---- K tail: perm + rope (overlaps Q0 proj on PE) ----
                p2k, free_p2k = tc.tile([128, U], f32, space="PSUM", name="p2k")
                for n0, nw in NB:
                    nc.tensor.matmul(
                        p2k[:, n0 : n0 + nw], perm_sb[:], p1k_sb[:, n0 : n0 + nw],
                        start=True, stop=True,
                    )
                p2k_sb = pin.tile([128, U], f32r, tag="p2ks")
                nc.vector.tensor_copy(p2k_sb[:], p2k[:])
                free_p2k()
                nc.vector.tensor_tensor(p1k_sb[:], p1k_sb[:], ck_sb[:], MULT)
                nc.vector.tensor_tensor(p2k_sb[:], p2k_sb[:], sk_sb[:], MULT)
                nc.vector.tensor_tensor(ktr[:], p1k_sb[:], p2k_sb[:], ADD)

                # ==== interleaved Q projection + phase B heads ====
                p1sb0 = q_copy(0, p1_t0)
                free_q0p1()

                with tile_ctx(tc, "pattn", 1) as pattn:
                    attn = [pattn.tile([128, SL], bf16, tag=f"attn{t}", name=f"attn{t}") for t in range(8)]
                    wo_sb = pattn.tile([128, 8192], bf16, tag="wo")
                    biasrep = pattn.tile([128, 1024], f32r, tag="brep")
                    with (
                        tc.tile_pool(name="spool", bufs=2, space="PSUM") as spool,
                        tc.tile_pool(name="ppool", bufs=6) as ppool,
                        tc.tile_pool(name="npool", bufs=4) as npool,
                        tc.tile_pool(name="ppv", bufs=2, space="PSUM") as ppv,
                    ):
                        # bias-replica for phase C (built once on PE + act)
                        br_ps = ppv.tile([128, 512], f32, tag="pv")
                        for nb2 in range(2):
                            nc.tensor.matmul(
                                br_ps[:], ones[0:1, 0:128],
                                bo_sb[0:1, 512 * nb2 : 512 * nb2 + 512],
                                start=True, stop=True,
                            )
                            nc.scalar.copy(biasrep[:, 512 * nb2 : 512 * nb2 + 512], br_ps[:])

                        deferred = []

                        def emit_head(h):
                            # qm[t] hosts heads (t, t+8): a head's 64 query lanes
                            # sit at partition base 64*kv, matching ktr's kv rows
                            t, kv = h % 8, h // 8
                            r0 = 64 * kv
                            at, ar0 = h // 2, 64 * (h % 2)  # attn rows for head h
                            for fn in deferred:
                                fn()
                            del deferred[:]
                            pts = {}
                            pv_ps = [ppv.tile([128, 512], f32, tag="pv", name=f"pv{h}_{m}")
                                     for m in range(2)]

                            def sc(p):
                                sp = spool.tile([128, 1024], f32, tag="sc")
                                for half in range(2):
                                    c = 2 * p + half
                                    nc.tensor.matmul(
                                        sp[:, 512 * half : 512 * half + 384],
                                        ktr[64 * kv : 64 * kv + 64, 128 * c : 128 * c + 128],
                                        qm[t][r0 : r0 + 64, 128 * c : 128 * c + 384],
                                        start=True, stop=True,
                                    )
                                pt = ppool.tile([128, 768], bf16, tag="pt")
                                nc.scalar.activation(
                                    pt[:].rearrange("p (b x) -> p b x", b=2),
                                    sp[:].rearrange("p (b x) -> p b x", b=2)[:, :, 0:384],
                                    Exp,
                                )
                                nc.vector.tensor_tensor(pt[:], pt[:], band[:], MULT)
                                pts[p] = pt

                            def pv(j):
                                m, sl8 = (j - 1) // 4, 128 * ((j - 1) % 4)
                                for c in (j - 1, j, j + 1):
                                    o = 384 * (c % 2) + 128 * (j - c + 1)
                                    nc.tensor.matmul(
                                        pv_ps[m][0:65, sl8 : sl8 + 128],
                                        vaug[kv][:, 65 * c : 65 * c + 65],
                                        pts[c // 2][:, o : o + 128],
                                        start=(c == j - 1),
                                        stop=(c == j + 1),
                                    )

                            def den_copy(m):
                                # denominator row (PV of the ones-column) -> SBUF
                                rd = npool.tile([1, 512], f32r, tag="rd")
                                nc.gpsimd.tensor_copy(rd[0:1, :], pv_ps[m][64:65, 0:512])
                                return rd

                            def rb_div(m, rd, pv_t, a, a0):
                                # broadcast den into unused partitions 64:128 of
                                # the pv psum tile, then divide on Pool
                                nc.tensor.matmul(
                                    pv_t[64:128, 0:512], ones[0:1, 0:64], rd[0:1, :],
                                    start=True, stop=True,
                                )
                                nc.gpsimd.tensor_tensor(
                                    attn[a][a0 : a0 + 64, 512 * m : 512 * m + 512],
                                    pv_t[0:64, 0:512],
                                    pv_t[64:128, 0:512],
                                    DIV,
                                )

                            sc(0); sc(1)
                            pv(1); pv(2)
                            sc(2)
                            pv(3); pv(4)
                            rd0 = den_copy(0)
                            sc(3)
                            pv(5); pv(6)
                            rb_div(0, rd0, pv_ps[0], at, ar0)
                            sc(4)
                            pv(7); pv(8)
                            rd1 = den_copy(1)
                            deferred.append(
                                lambda m=1, rd=rd1, pv_t=pv_ps[1], a=at, a0=ar0:
                                    rb_div(m, rd, pv_t, a, a0)
                            )

                        def p2_single():
                            p2, free_p2 = tc.tile([128, SL], f32, space="PSUM", name="p2s_")
                            return p2, free_p2

                        # t=0 tail, then per-t: proj+tail then one interleaved head
                        p2, free_p2 = p2_single()
                        q_tail(0, p1sb0, lambda: p2)
                        free_p2()
                        for t in range(1, 8):
                            p1, free_p1 = tc.tile([128, SL], f32, space="PSUM", name="p1s_")
                            q_proj_mms(t, p1)
                            p1_sb = q_copy(t, p1)
                            free_p1()
                            p2, free_p2 = p2_single()
                            q_tail(t, p1_sb, lambda: p2)
                            free_p2()
                            emit_head(t - 1)
                            if t == 4:
                                # wo loads (input DMAs are long done; C is far off)
                                for i in range(8):
                                    q = nc.sync if i % 2 else nc.gpsimd
                                    q.dma_start(
                                        wo_sb[:, 1024 * i : 1024 * i + 1024],
                                        wom[128 * i : 128 * i + 128, :],
                                    )
                        for h in range(7, H):
                            emit_head(h)
                        for fn in deferred:
                            fn()

                    # ================= phase C: output projection =================
                    with (
                        tc.tile_pool(name="oout", bufs=3) as pou,
                        tc.tile_pool(name="ops", bufs=2, space="PSUM") as ops,
                    ):
                        for tq in range(8):
                            q0 = 128 * tq
                            for nb2 in range(2):
                                op = ops.tile([128, 512], f32, tag="op")
                                for kc in range(8):
                                    nc.tensor.matmul(
                                        op[:],
                                        attn[kc][:, q0 : q0 + 128],
                                        wo_sb[:, 1024 * kc + 512 * nb2 : 1024 * kc + 512 * nb2 + 512],
                                        start=(kc == 0), stop=(kc == 7),
                                    )
                                ot = pou.tile([128, 512], f32, tag="ot")
                                nc.vector.tensor_tensor(
                                    ot[:], op[:], biasrep[:, 512 * nb2 : 512 * nb2 + 512], ADD
                                )
                                nc.sync.dma_start(
                                    out[q0 : q0 + 128, 512 * nb2 : 512 * nb2 + 512], ot[:]
                                )
    nc.finalize()
    return nc


# Q columns: qm[t] hosts heads (t, t+8); per head: [even lanes] + [odd lanes]
_HEAD_ORDER = [t + 8 * p for t in range(8) for p in range(2)]
_PERM_QM = np.concatenate(
    [np.concatenate([64 * h + 2 * np.arange(32), 64 * h + 2 * np.arange(32) + 1])
     for h in _HEAD_ORDER]
)
# K columns: for kv in 0,1: [64kv+2i] + [64kv+2i+1]
_PERM_KM = np.concatenate(
    [np.concatenate([64 * kv + 2 * np.arange(32), 64 * kv + 2 * np.arange(32) + 1])
     for kv in range(KVH)]
)
# 32-row pair-swap permutation (i <-> i^32)
_PERM128 = np.zeros((128, 128), np.float32)
_PERM128[np.arange(128), np.arange(128) ^ 32] = 1.0
# sign pattern for the S' rope tile: -1 on even 32-row groups, +1 on odd
_SGN = np.repeat(np.array([-1.0, 1.0, -1.0, 1.0], np.float32), 32)[:, None]


def _chunk_major(w):
    # [D, F] -> [128, 8*F]: column block kc holds rows 128kc..128kc+127
    F = w.shape[1]
    return np.ascontiguousarray(w.reshape(8, 128, F).transpose(1, 0, 2).reshape(128, 8 * F))


def make_inputs(x, freqs_cis, w_q, b_q, w_k, b_k, w_v, b_v, w_o, b_o):
    import ml_dtypes

    BF = ml_dtypes.bfloat16
    cos = np.asarray(freqs_cis[..., 0], dtype=np.float32)  # (S, 32)
    sin = np.asarray(freqs_cis[..., 1], dtype=np.float32)
    x = np.asarray(x, dtype=np.float32)
    band0 = np.zeros((128, 384), np.float32)
    for k in range(128):
        band0[k, k + 1 : k + 256] = 1.0
    bandm = np.concatenate([band0, band0], axis=1)
    # wq: [128, 8192] tile-major: cols 1024t+128kc+j = w[128kc+p, 128t+j]
    wq_p = w_q[:, _PERM_QM]
    wqm = wq_p.reshape(8, 128, 8, 128).transpose(1, 2, 0, 3).reshape(128, 8192)
    common = dict(
        wqm=np.ascontiguousarray(wqm).astype(BF),
        wkm=_chunk_major(w_k[:, _PERM_KM]).astype(BF),
        wvm=_chunk_major(np.asarray(w_v)).astype(BF),
        wom=np.ascontiguousarray(w_o).astype(BF),
        permm=_PERM128.astype(np.float32),
        bandm=bandm.astype(BF),
        bqc=np.ascontiguousarray(b_q[_PERM_QM].reshape(8, 128).T).astype(np.float32),
        bkc=np.asarray(b_k[_PERM_KM], np.float32)[:, None],
        bvc=np.asarray(b_v, np.float32)[:, None],
        boc=np.asarray(b_o, np.float32)[None, :],
    )
    maps = []
    for c in range(8):
        b, hf = c // 2, c % 2
        s0 = SL * hf
        pos = s0 - PAD + np.arange(U)
        valid = (pos >= 0) & (pos < S)
        pc = np.clip(pos, 0, S - 1)
        xTc = np.where(valid[None, :], x[b][pc].T, 0.0).astype(BF)
        ckc = np.tile(cos[pc].T, (4, 1)).astype(np.float32)
        skc = (np.tile(sin[pc].T, (4, 1)) * _SGN).astype(np.float32)
        qpos = s0 + np.arange(SL)
        cqc = np.tile(cos[qpos].T, (4, 1)).astype(np.float32)
        sqc = (np.tile(sin[qpos].T, (4, 1)) * _SGN).astype(np.float32)
        m = dict(common)
        m.update(xT=xTc, cqt=cqc, sqt=sqc, ckt=ckc, skt=skc)
        maps.append(m)
    return maps


_NC_CACHE = {}


def kernel(x, freqs_cis, w_q, b_q, w_k, b_k, w_v, b_v, w_o, b_o):
    if "nc" not in _NC_CACHE:
        _NC_CACHE["nc"] = build_nc()
    nc = _NC_CACHE["nc"]
    maps = make_inputs(
        np.asarray(x), np.asarray(freqs_cis), np.asarray(w_q), np.asarray(b_q),
        np.asarray(w_k), np.asarray(b_k), np.asarray(w_v), np.asarray(b_v),
        np.asarray(w_o), np.asarray(b_o),
    )
    res = run_bass_kernel_spmd(nc, maps, list(range(8))).results
    full = np.empty((B, S, D), np.float32)
    for c in range(8):
        b, hf = c // 2, c % 2
        full[b, SL * hf : SL * (hf + 1), :] = res[c]["out"]
    return full
